# revision 10
# baseline (speedup 1.0000x reference)
"""Single-head causal attention with RoPE + padding mask, data-parallel
over batch across 8 TRN2 NeuronCores (one batch element per core).

Per core (T=4096, C=128, HS=64):
  q = rope(x @ Wq); k = rope(x @ Wk); v = x @ Wv
  S^T[j,i] = k[j]·q[i]           (scores, transposed layout: partition=j)
  P^T = exp(S^T/sqrt(C)) * tri(i>=j)
  outT[d,i] = sum_j (mask[j]*v[j,d]) P^T[j,i]; rowsum via a mask column
        appended to v.  final out[i,d] = outT[d,i]/rowsum[i] on HOST.

Two-engine exp: ScalarE ACTIVATE handles the diagonal-band groups
(exact exp + gpsimd triangle select) plus a balanced share of the far
groups; VectorE handles the rest via a one-instruction Schraudolph
approximation: int16(bits) = rint(s*A + B) interpreted as bf16 --
|rel err| <= 3.1% per weight, harmless for rows >= 129 keys (error
averages over many keys; only diagonal-band rows are few-key).

Load balancing: rope muls are forced DVE (PSUM source); the rope adds
(SBUF-only) split DVE/GPSIMD; v-scale and output copies split ACT/DVE
(ACT runs them as activation-Copy with per-partition scale).
Projections run 3 chunks ahead of the score pipeline so the
proj->rope->scores latency never starves the PE (HAM re-throttles the
PE clock to 1.2GHz after ~3.4us of low activity -- LDWEIGHTS burns
bridge the thin early chunks).
"""

import numpy as np
import os

T, C, HS = 4096, 128, 64
N_CORES = 8
NT = T // 128      # 32 j-tiles of 128
NCH = T // 512     # 8 i-chunks of 512
SCALE = float(1.0 / np.sqrt(np.float32(C)))
# Schraudolph bf16-bits exp: bits = rint(s*EA + EB); value = bf16(bits)
EA = float(128.0 * SCALE * np.log2(np.e))
EB = float(16256.0 - 5.35)
# fp8e4m3-bits exp for the fp8 PV path: bits8 = rint(s*EA8 + EB8)
EA8 = float(8.0 * SCALE * np.log2(np.e))
EB8 = float(56.0 - 0.36)

_CACHE = {}


def _install_tile_drain_patch(tile_mod):
    """This container's walrus rejects instructions with >2 sem waits; split
    Tile's final global drain into one drain per ticked processor."""
    import bass_rust
    from concourse.vector_clock import ScopedClock

    def _patched(self, tick_clock, wait_clock):
        gc = tick_clock.global_clock
        for i in range(len(gc)):
            if gc[i] <= 0:
                continue
            v = bass_rust.VectorClock()
            v.require_at_least(i, gc[i])
            d = self.nc.sync.drain()
            wait_clock.add_sem_waits(d.ins, ScopedClock({None: v}))
        self.nc.all_engine_barrier()
        assert self.sems is not None
        popped = self.nc._tile_sem_poison_stack.pop()
        assert popped is self._sem_poison
        self.nc.clear_and_free_semaphores(list(self.sems.allocated().values()))
        self.nc.all_engine_barrier()

    tile_mod.TileContext._drain_and_barrier = _patched


def _split_excess_waits(nc, mybir, limit=1):
    """This container's walrus rejects instructions with >limit sem waits.
    Hoist excess waits onto standalone EventSemaphore instructions inserted
    just before the offending instruction on the same engine queue."""
    ctr = 0
    for f in nc.m.functions:
        for b in f.blocks:
            il = b.instructions
            out = []
            changed = False
            for ins in il:
                si = ins.sync_info
                waits = list(si.on_wait) if si and si.on_wait else []
                if len(waits) > limit:
                    changed = True
                    excess = waits[: len(waits) - limit]
                    keep = waits[len(waits) - limit :]
                    for i in range(0, len(excess), limit):
                        chunk = excess[i : i + limit]
                        ev = mybir.InstEventSemaphore(
                            name=f"I-waitsplit-{ctr}",
                            engine=ins.engine,
                            ins=[],
                            outs=[],
                            sync_info=mybir.SyncInfo(on_wait=chunk, on_update=[]),
                        )
                        ctr += 1
                        nc.register_instruction(ev)
                        out.append(ev)
                    si.on_wait = keep
                out.append(ins)
            if changed:
                b.instructions = out
    return nc


def _groups_for_chunk(ic):
    """Group layout for i-chunk ic: far full-width pairs + 2 diag groups.
    Entry: (jt, i_lo, sg_off, width, diag_off, row_group)."""
    groups = []
    for p in range(0, 4 * ic, 2):
        groups.append(
            [(p, 0, 0, 512, None, 0), (p + 1, 0, 512, 512, None, 64)]
        )
    b = 4 * ic
    groups.append([(b + 0, 0, 0, 512, 0, 0), (b + 2, 256, 512, 256, 512, 64)])
    groups.append([(b + 1, 128, 0, 384, 0, 0), (b + 3, 384, 384, 128, 384, 0)])
    return groups


class _Balancer:
    """Greedy multi-queue load balancer (estimated ns per op)."""

    def __init__(self):
        self.act = 0.0
        self.dve = 0.0
        self.gp = 0.0

    def pick2(self, attr_costs):
        # attr_costs: list of (attr, cost); choose min completion time
        best = min(attr_costs, key=lambda ac: getattr(self, ac[0]) + ac[1])
        setattr(self, best[0], getattr(self, best[0]) + best[1])
        return best[0]


def _build_nc():
    import concourse.bass as bass
    import concourse.mybir as mybir
    from concourse import tile

    _install_tile_drain_patch(tile)

    DT = mybir.dt
    F32, BF16, I16 = DT.float32, DT.bfloat16, DT.int16
    I8, F8 = DT.int8, DT.float8e4
    PM = mybir.MatmulPerfMode
    AF = mybir.ActivationFunctionType
    ALU = mybir.AluOpType

    nc = bass.Bass()
    xT_e = nc.declare_dram_parameter("p_xt", [C, T], BF16, isOutput=False)
    w_e = nc.declare_dram_parameter("p_w", [C, 576], BF16, isOutput=False)
    cosx_e = nc.declare_dram_parameter("p_cos", [128, T], BF16, isOutput=False)
    sinx_e = nc.declare_dram_parameter("p_sin", [128, T], BF16, isOutput=False)
    mask01_e = nc.declare_dram_parameter("p_mask", [128, NT], F32, isOutput=False)
    out_e = nc.declare_dram_parameter("p_out", [HS + 1, T], F32, isOutput=True)

    bal = _Balancer()
    C_GRP_A = lambda fd: (fd + 352) / 1.2 + 110
    C_GRP_D = lambda fd: (fd + 120) / 0.96 + 170
    C_V_A = (64 + 352) / 1.2 + 110
    C_V_D = (64 + 120) / 0.96 + 80
    C_OSB_A = (512 + 352) / 1.2 + 110
    C_OSB_D = (512 + 120) / 0.96 + 80
    C_MUL_D = 685 + 60
    C_ADD_D = 424 + 80
    C_ADD_G = 1109 + 200
    C_AFF_G = 400.0
    C_VF8_G = 220.0
    C_VF8_D = 174.0
    C_DMA_G = 780.0
    ldw_burn = not os.environ.get("KERNEL_NO_LDWBURN")

    with tile.TileContext(nc) as tc:
        with (
            tc.tile_pool(name="const", bufs=1) as cpool,
            tc.tile_pool(name="work", bufs=3) as wpool,
            tc.tile_pool(name="ps", bufs=2, space="PSUM") as ps,
        ):
            xT = cpool.tile([C, T], BF16)
            w_sb = cpool.tile([C, 576], BF16)
            mask01 = cpool.tile([128, NT], F32)
            cosx = cpool.tile([128, T], BF16)
            sinx = cpool.tile([128, T], BF16)

            # preload the exp activation table while input DMAs run
            warm = cpool.tile([128, 8], F32)
            nc.vector.memset(warm[:, :], 0.0)
            nc.scalar.activation(warm[:, :], warm[:, :], AF.Exp, bias=0.0, scale=1.0)

            def _sl(ch):
                return slice(ch * 512, (ch + 1) * 512)

            # head-critical input DMAs, one per queue, then big batched tails
            nc.sync.dma_start(out=w_sb[:, :], in_=w_e[:, :])
            nc.gpsimd.dma_start(out=xT[:, _sl(0)], in_=xT_e[:, _sl(0)])
            nc.scalar.dma_start(out=cosx[:, _sl(0)], in_=cosx_e[:, _sl(0)])
            nc.sync.dma_start(out=sinx[:, _sl(0)], in_=sinx_e[:, _sl(0)])
            nc.gpsimd.dma_start(out=xT[:, _sl(1)], in_=xT_e[:, _sl(1)])
            nc.scalar.dma_start(out=cosx[:, _sl(1)], in_=cosx_e[:, _sl(1)])
            nc.sync.dma_start(out=sinx[:, _sl(1)], in_=sinx_e[:, _sl(1)])
            nc.gpsimd.dma_start(out=mask01[:, :], in_=mask01_e[:, :])
            bal.gp += 3 * C_DMA_G
            rest = slice(2 * 512, T)
            nc.gpsimd.dma_start(out=xT[:, rest], in_=xT_e[:, rest])
            bal.gp += C_DMA_G
            nc.scalar.dma_start(out=cosx[:, rest], in_=cosx_e[:, rest])
            nc.sync.dma_start(out=sinx[:, rest], in_=sinx_e[:, rest])

            # HAM warm-up burns: PE busy from ~0.3us
            scratch = cpool.tile([128, 512], BF16)
            nc.vector.memset(scratch[:, :], 0.0)
            burn = ps.tile([80, 512], F32, tag="outT", bufs=2, name="burn")
            for _ in range(10):
                nc.tensor.matmul(
                    burn[:, :], scratch[:, 0:80], scratch[:, :],
                    start=True, stop=True,
                )

            q2 = cpool.tile([128, T], BF16)
            k2 = cpool.tile([128, T], BF16)

            # v tiles + mask column: [t, j_tile, 65]
            vplus = cpool.tile([128, NT, HS + 1], BF16)
            nc.vector.tensor_copy(vplus[:, :, HS], mask01[:, :])
            # fp8 interleaved-pair copy of v for DoubleRow PV (80-col padded
            # stationary so the Ko step is 16B-aligned; pad stays zero)
            vplus_f8 = cpool.tile([128, NT // 2, 2, 80], F8)
            nc.vector.memset(vplus_f8[:, :, :, :], 0.0)

            def rope_finish(dst, sl, m1, m2):
                if bal.pick2([("dve", C_ADD_D), ("gp", C_ADD_G)]) == "dve":
                    nc.vector.tensor_add(dst[:, sl], m1[:, :], m2[:, :])
                else:
                    nc.gpsimd.tensor_tensor(
                        dst[:, sl], m1[:, :], m2[:, :], ALU.add
                    )

            def proj_q(ch):
                sl = slice(ch * 512, (ch + 1) * 512)
                raw = ps.tile([128, 512], F32, tag="proj", bufs=2, name=f"qr{ch}")
                nc.tensor.matmul(raw[:, :], w_sb[:, 0:128], xT[:, sl], start=True, stop=True)
                swp = ps.tile([128, 512], F32, tag="proj", bufs=2, name=f"qs{ch}")
                nc.tensor.matmul(swp[:, :], w_sb[:, 128:256], xT[:, sl], start=True, stop=True)
                bal.dve += 2 * C_MUL_D
                m1 = wpool.tile([128, 512], BF16, tag="rope", bufs=4, name=f"m1_{ch}")
                nc.vector.tensor_mul(m1[:, :], raw[:, :], cosx[:, sl])
                m2 = wpool.tile([128, 512], BF16, tag="rope", bufs=4, name=f"m2_{ch}")
                nc.vector.tensor_mul(m2[:, :], swp[:, :], sinx[:, sl])
                rope_finish(q2, sl, m1, m2)

            def proj_k(ch):
                sl = slice(ch * 512, (ch + 1) * 512)
                raw = ps.tile([128, 512], F32, tag="proj", bufs=2, name=f"kr{ch}")
                nc.tensor.matmul(raw[:, :], w_sb[:, 256:384], xT[:, sl], start=True, stop=True)
                swp = ps.tile([128, 512], F32, tag="proj", bufs=2, name=f"ks{ch}")
                nc.tensor.matmul(swp[:, :], w_sb[:, 384:512], xT[:, sl], start=True, stop=True)
                bal.dve += 2 * C_MUL_D
                m3 = wpool.tile([128, 512], BF16, tag="rope", bufs=4, name=f"m3_{ch}")
                nc.vector.tensor_mul(m3[:, :], raw[:, :], cosx[:, sl])
                m4 = wpool.tile([128, 512], BF16, tag="rope", bufs=4, name=f"m4_{ch}")
                nc.vector.tensor_mul(m4[:, :], swp[:, :], sinx[:, sl])
                rope_finish(k2, sl, m3, m4)

            def v_block(ch):
                for tt in range(4):
                    jt = ch * 4 + tt
                    v_ps = ps.tile([128, HS], F32, tag="proj", bufs=2, name=f"v{jt}")
                    nc.tensor.matmul(
                        v_ps[:, :],
                        xT[:, jt * 128 : (jt + 1) * 128],
                        w_sb[:, 512:576],
                        start=True,
                        stop=True,
                    )
                    if bal.pick2([("act", C_V_A), ("dve", C_V_D)]) == "act":
                        nc.scalar.activation(
                            vplus[:, jt, 0:HS], v_ps[:, :], AF.Copy,
                            bias=0.0, scale=mask01[:, jt : jt + 1],
                        )
                    else:
                        nc.vector.tensor_scalar_mul(
                            vplus[:, jt, 0:HS], v_ps[:, :], mask01[:, jt : jt + 1]
                        )
                    if bal.pick2([("gp", C_VF8_G), ("dve", C_VF8_D)]) == "gp":
                        nc.gpsimd.tensor_copy(
                            vplus_f8[:, jt // 2, jt % 2, 0 : HS + 1], vplus[:, jt, :]
                        )
                    else:
                        nc.vector.tensor_copy(
                            vplus_f8[:, jt // 2, jt % 2, 0 : HS + 1], vplus[:, jt, :]
                        )

            # deep projection head: 3 chunks ahead
            proj_q(0)
            proj_k(0)
            proj_q(1)
            proj_k(1)
            v_block(0)
            proj_q(2)
            proj_k(2)
            v_block(1)

            # flat group list with background projection work attached
            work = []  # (ic, group, is_last_of_chunk, bg_blocks)
            for ic in range(NCH):
                gs = _groups_for_chunk(ic)
                for gi, g in enumerate(gs):
                    bg = []
                    if gi == 0 and ic + 3 < NCH:
                        bg.append(("pq", ic + 3))
                    if gi == min(1, len(gs) - 1) and ic + 3 < NCH:
                        bg.append(("pk", ic + 3))
                    if gi == min(2, len(gs) - 1) and 2 <= ic + 2 < NCH:
                        bg.append(("v", ic + 2))
                    work.append((ic, g, gi == len(gs) - 1, bg))

            def emit_scores(ic, g, sg):
                isl0 = ic * 512
                for jt, i_lo, off, w, _d, ro in g:
                    nc.tensor.matmul(
                        sg[:, off : off + w],
                        k2[ro : ro + HS, jt * 128 : (jt + 1) * 128],
                        q2[ro : ro + HS, isl0 + i_lo : isl0 + 512],
                        start=True,
                        stop=True,
                        tile_position=(ro, 0),
                    )

            def emit_pv(ic, g, pt, outT):
                njt = 4 * (ic + 1)
                diag = any(e[4] is not None for e in g)
                if not diag:
                    # far pair -> one fp8 DoubleRow matmul (K=256)
                    pair = g[0][0] // 2
                    nc.tensor.matmul(
                        outT[:, 0:512],
                        vplus_f8[:, pair, :, :],
                        pt[:, 0:1024].rearrange("p (r n) -> p r n", r=2),
                        start=(g[0][0] == 0),
                        stop=False,
                        perf_mode=PM.DoubleRow,
                        skip_group_check=True,
                    )
                    return
                for jt, i_lo, off, w, _d, _ro in g:
                    nc.tensor.matmul(
                        outT[0 : HS + 1, i_lo:512],
                        vplus[:, jt, :],
                        pt[:, off : off + w],
                        start=(jt == 0),
                        stop=(jt == njt - 1),
                        skip_group_check=True,
                    )

            def emit_out(pic, pouT):
                lo = 128 if pic == NCH - 1 else 0  # chunk 7 [0:128] shipped early
                osb = wpool.tile([HS + 1, 512], F32, tag="osb", bufs=2, name=f"osb{pic}")
                if bal.pick2([("act", C_OSB_A), ("dve", C_OSB_D)]) == "act":
                    nc.scalar.activation(
                        osb[:, lo:512], pouT[0 : HS + 1, lo:512], AF.Copy, bias=0.0, scale=1.0
                    )
                else:
                    nc.vector.tensor_copy(osb[:, lo:512], pouT[0 : HS + 1, lo:512])
                o0 = pic * 512
                nc.sync.dma_start(out=out_e[:, o0 + lo : o0 + 256], in_=osb[:, lo:256])
                nc.gpsimd.dma_start(out=out_e[:, o0 + 256 : o0 + 512], in_=osb[:, 256:512])
                bal.gp += C_DMA_G

            pending = None  # (ic, group, pt, outT, last?)
            outT_cur = None
            gidx = 0
            for ic, g, last, bg in work:
                diag = any(e[4] is not None for e in g)
                fd = g[-1][2] + g[-1][3]
                if g[0][0] == 0:
                    outT_cur = ps.tile(
                        [80, 512], F32, tag="outT", bufs=2, name=f"oT{ic}"
                    )
                sg = ps.tile([128, 1024], F32, tag="sg", bufs=2, name=f"sg{ic}_{g[0][0]}")
                emit_scores(ic, g, sg)
                # HAM hold: LDWEIGHTS-only burns keep the PE activity monitor
                # fed through the thin early chunks (no PSUM side effects)
                if ldw_burn and gidx < 10:
                    for _ in range(2):
                        nc.tensor.ldweights(weights=scratch[:, 0:128])
                gidx += 1
                if pending is not None:
                    pic, pg, ppt, pouT, plast = pending
                    emit_pv(pic, pg, ppt, pouT)
                    if plast:
                        emit_out(pic, pouT)
                    elif pic == NCH - 1 and pg[0][4] is not None:
                        osb7 = wpool.tile(
                            [HS + 1, 128], F32, tag="osb7", bufs=1, name="osb7p"
                        )
                        nc.vector.tensor_copy(osb7[:, :], pouT[0 : HS + 1, 0:128])
                        nc.sync.dma_start(
                            out=out_e[:, pic * 512 : pic * 512 + 128],
                            in_=osb7[:, :],
                        )
                # background projections first: DVE queue serves the rope
                # muls (producer-critical) before a long exp
                for kind, cc in bg:
                    if kind == "pq":
                        proj_q(cc)
                    elif kind == "pk":
                        proj_k(cc)
                    else:
                        v_block(cc)
                if diag:
                    pt = wpool.tile(
                        [128, 1024], BF16, tag="ptd", bufs=2,
                        name=f"pt{ic}_{g[0][0]}",
                    )
                    bal.act += C_GRP_A(fd)
                    nc.scalar.activation(
                        pt[:, 0:fd], sg[:, 0:fd], AF.Exp, bias=0.0, scale=SCALE
                    )
                else:
                    pt = wpool.tile(
                        [128, 1024], F8, tag="pt", bufs=4,
                        name=f"pt{ic}_{g[0][0]}",
                    )
                    eng = bal.pick2([("act", C_GRP_A(fd)), ("dve", C_GRP_D(fd))])
                    if eng == "act":
                        nc.scalar.activation(
                            pt[:, 0:fd], sg[:, 0:fd], AF.Exp, bias=0.0, scale=SCALE
                        )
                    else:
                        nc.vector.tensor_scalar(
                            pt[:, 0:fd].bitcast(I8), sg[:, 0:fd], EA8, EB8,
                            ALU.mult, ALU.add,
                        )
                for _jt, _i_lo, _off, _w, d, _ro in g:
                    if d is not None:
                        bal.gp += C_AFF_G
                        nc.gpsimd.affine_select(
                            out=pt[:, d : d + 128],
                            in_=pt[:, d : d + 128],
                            compare_op=ALU.is_ge,
                            fill=0.0,
                            base=0,
                            pattern=[[1, 128]],
                            channel_multiplier=-1,
                        )
                pending = (ic, g, pt, outT_cur, last)

            pic, pg, ppt, pouT, plast = pending
            emit_pv(pic, pg, ppt, pouT)
            emit_out(pic, pouT)

    import concourse.mybir as mybir
    _split_excess_waits(nc, mybir, limit=1)
    if os.environ.get("KERNEL_DEBUG_BAL"):
        print(f"balancer: act={bal.act/1000:.1f}us dve={bal.dve/1000:.1f}us gp={bal.gp/1000:.1f}us")
    return nc


def _get_nc():
    if "nc" not in _CACHE:
        _CACHE["nc"] = _build_nc()
    return _CACHE["nc"]


def kernel(x_text_emb, Wq, Wk, Wv, freqs_cos, freqs_sin, x_latex_mask):
    import ml_dtypes
    from concourse.bass_utils import run_bass_kernel_spmd

    bf16 = ml_dtypes.bfloat16
    nc = _get_nc()

    swap = np.arange(HS) ^ 1
    cos2 = np.repeat(np.asarray(freqs_cos, np.float32).T, 2, axis=0)  # [64, T]
    sin2s = np.repeat(np.asarray(freqs_sin, np.float32).T, 2, axis=0)
    sin2s[0::2] *= -1.0
    cosx = np.ascontiguousarray(np.tile(cos2, (2, 1))).astype(bf16)  # [128, T]
    sinx = np.ascontiguousarray(np.tile(sin2s, (2, 1))).astype(bf16)
    Wq = np.asarray(Wq, np.float32)
    Wk = np.asarray(Wk, np.float32)
    Wv = np.asarray(Wv, np.float32)
    w = np.concatenate(
        [Wq, Wq, Wq[:, swap], Wq[:, swap], Wk, Wk, Wk[:, swap], Wk[:, swap], Wv],
        axis=1,
    ).astype(bf16)
    w = np.ascontiguousarray(w)
    mask01 = np.asarray(x_latex_mask != 0, np.float32).reshape(N_CORES, NT, 128)

    in_maps = []
    for b in range(N_CORES):
        in_maps.append(
            {
                "p_xt": np.ascontiguousarray(
                    np.asarray(x_text_emb[b], np.float32).T
                ).astype(bf16),
                "p_w": w,
                "p_cos": cosx,
                "p_sin": sinx,
                "p_mask": np.ascontiguousarray(mask01[b].T),
            }
        )

    res = run_bass_kernel_spmd(nc, in_maps, core_ids=list(range(N_CORES)))
    outs = []
    for b in range(N_CORES):
        r = np.asarray(res.results[b]["p_out"], np.float32)
        outs.append((r[0:HS, :] / r[HS : HS + 1, :]).T)
    return np.stack(outs, axis=0)


# revision 11
# speedup vs baseline: 1.0136x; 1.0136x over previous
"""Single-head causal attention with RoPE + padding mask, data-parallel
over batch across 8 TRN2 NeuronCores (one batch element per core).

Per core (T=4096, C=128, HS=64):
  q = rope(x @ Wq); k = rope(x @ Wk); v = x @ Wv
  S^T[j,i] = k[j]·q[i]           (scores, transposed layout: partition=j)
  P^T = exp(S^T/sqrt(C)) * tri(i>=j)
  outT[d,i] = sum_j (mask[j]*v[j,d]) P^T[j,i]; rowsum via a mask column
        appended to v.  final out[i,d] = outT[d,i]/rowsum[i] on HOST.

Two-engine exp: ScalarE ACTIVATE handles the diagonal-band groups
(exact exp + gpsimd triangle select) plus a balanced share of the far
groups; VectorE handles the rest via a one-instruction Schraudolph
approximation: int16(bits) = rint(s*A + B) interpreted as bf16 --
|rel err| <= 3.1% per weight, harmless for rows >= 129 keys (error
averages over many keys; only diagonal-band rows are few-key).

Load balancing: rope muls are forced DVE (PSUM source); the rope adds
(SBUF-only) split DVE/GPSIMD; v-scale and output copies split ACT/DVE
(ACT runs them as activation-Copy with per-partition scale).
Projections run 3 chunks ahead of the score pipeline so the
proj->rope->scores latency never starves the PE (HAM re-throttles the
PE clock to 1.2GHz after ~3.4us of low activity -- LDWEIGHTS burns
bridge the thin early chunks).
"""

import numpy as np
import os

T, C, HS = 4096, 128, 64
N_CORES = 8
NT = T // 128      # 32 j-tiles of 128
NCH = T // 512     # 8 i-chunks of 512
SCALE = float(1.0 / np.sqrt(np.float32(C)))
# Schraudolph bf16-bits exp: bits = rint(s*EA + EB); value = bf16(bits)
EA = float(128.0 * SCALE * np.log2(np.e))
EB = float(16256.0 - 5.35)
# fp8e4m3-bits exp for the fp8 PV path: bits8 = rint(s*EA8 + EB8)
EA8 = float(8.0 * SCALE * np.log2(np.e))
EB8 = float(56.0 - 0.36)

_CACHE = {}


def _install_tile_drain_patch(tile_mod):
    """This container's walrus rejects instructions with >2 sem waits; split
    Tile's final global drain into one drain per ticked processor."""
    import bass_rust
    from concourse.vector_clock import ScopedClock

    def _patched(self, tick_clock, wait_clock):
        gc = tick_clock.global_clock
        for i in range(len(gc)):
            if gc[i] <= 0:
                continue
            v = bass_rust.VectorClock()
            v.require_at_least(i, gc[i])
            d = self.nc.sync.drain()
            wait_clock.add_sem_waits(d.ins, ScopedClock({None: v}))
        self.nc.all_engine_barrier()
        assert self.sems is not None
        popped = self.nc._tile_sem_poison_stack.pop()
        assert popped is self._sem_poison
        self.nc.clear_and_free_semaphores(list(self.sems.allocated().values()))
        self.nc.all_engine_barrier()

    tile_mod.TileContext._drain_and_barrier = _patched


def _split_excess_waits(nc, mybir, limit=1):
    """This container's walrus rejects instructions with >limit sem waits.
    Hoist excess waits onto standalone EventSemaphore instructions inserted
    just before the offending instruction on the same engine queue."""
    ctr = 0
    for f in nc.m.functions:
        for b in f.blocks:
            il = b.instructions
            out = []
            changed = False
            for ins in il:
                si = ins.sync_info
                waits = list(si.on_wait) if si and si.on_wait else []
                if len(waits) > limit:
                    changed = True
                    excess = waits[: len(waits) - limit]
                    keep = waits[len(waits) - limit :]
                    for i in range(0, len(excess), limit):
                        chunk = excess[i : i + limit]
                        ev = mybir.InstEventSemaphore(
                            name=f"I-waitsplit-{ctr}",
                            engine=ins.engine,
                            ins=[],
                            outs=[],
                            sync_info=mybir.SyncInfo(on_wait=chunk, on_update=[]),
                        )
                        ctr += 1
                        nc.register_instruction(ev)
                        out.append(ev)
                    si.on_wait = keep
                out.append(ins)
            if changed:
                b.instructions = out
    return nc


def _groups_for_chunk(ic):
    """Group layout for i-chunk ic: far full-width pairs + 2 diag groups.
    Entry: (jt, i_lo, sg_off, width, diag_off, row_group)."""
    groups = []
    for p in range(0, 4 * ic, 2):
        groups.append(
            [(p, 0, 0, 512, None, 0), (p + 1, 0, 512, 512, None, 64)]
        )
    b = 4 * ic
    groups.append([(b + 0, 0, 0, 512, 0, 0), (b + 2, 256, 512, 256, 512, 64)])
    groups.append([(b + 1, 128, 0, 384, 0, 0), (b + 3, 384, 384, 128, 384, 0)])
    return groups


class _Balancer:
    """Greedy multi-queue load balancer (estimated ns per op)."""

    def __init__(self):
        self.act = 0.0
        self.dve = 0.0
        self.gp = 0.0

    def pick2(self, attr_costs):
        # attr_costs: list of (attr, cost); choose min completion time
        best = min(attr_costs, key=lambda ac: getattr(self, ac[0]) + ac[1])
        setattr(self, best[0], getattr(self, best[0]) + best[1])
        return best[0]


def _build_nc():
    import concourse.bass as bass
    import concourse.mybir as mybir
    from concourse import tile

    _install_tile_drain_patch(tile)

    DT = mybir.dt
    F32, BF16, I16 = DT.float32, DT.bfloat16, DT.int16
    I8, F8 = DT.int8, DT.float8e4
    PM = mybir.MatmulPerfMode
    AF = mybir.ActivationFunctionType
    ALU = mybir.AluOpType

    nc = bass.Bass()
    xT_e = nc.declare_dram_parameter("p_xt", [C, T], BF16, isOutput=False)
    w_e = nc.declare_dram_parameter("p_w", [C, 576], BF16, isOutput=False)
    cosx_e = nc.declare_dram_parameter("p_cos", [128, T], BF16, isOutput=False)
    sinx_e = nc.declare_dram_parameter("p_sin", [128, T], BF16, isOutput=False)
    mask01_e = nc.declare_dram_parameter("p_mask", [128, NT], F32, isOutput=False)
    out_e = nc.declare_dram_parameter("p_out", [HS + 1, T], F32, isOutput=True)

    bal = _Balancer()
    C_GRP_A = lambda fd: (fd + 352) / 1.2 + 110
    C_GRP_D = lambda fd: (fd + 120) / 0.96 + 170
    C_V_A = (64 + 352) / 1.2 + 110
    C_V_D = (64 + 120) / 0.96 + 80
    C_OSB_A = (512 + 352) / 1.2 + 110
    C_OSB_D = (512 + 120) / 0.96 + 80
    C_MUL_D = 685 + 60
    C_ADD_D = 424 + 80
    C_ADD_G = 1109 + 200
    C_AFF_G = 400.0
    C_VF8_G = 220.0
    C_VF8_D = 174.0
    C_DMA_G = 780.0
    ldw_burn = not os.environ.get("KERNEL_NO_LDWBURN")

    with tile.TileContext(nc) as tc:
        with (
            tc.tile_pool(name="const", bufs=1) as cpool,
            tc.tile_pool(name="work", bufs=3) as wpool,
            tc.tile_pool(name="ps", bufs=2, space="PSUM") as ps,
        ):
            xT = cpool.tile([C, T], BF16)
            w_sb = cpool.tile([C, 576], BF16)
            mask01 = cpool.tile([128, NT], F32)
            cosx = cpool.tile([128, T], BF16)
            sinx = cpool.tile([128, T], BF16)

            # preload the exp activation table while input DMAs run
            warm = cpool.tile([128, 8], F32)
            nc.vector.memset(warm[:, :], 0.0)
            nc.scalar.activation(warm[:, :], warm[:, :], AF.Exp, bias=0.0, scale=1.0)

            def _sl(ch):
                return slice(ch * 512, (ch + 1) * 512)

            # head-critical input DMAs, one per queue, then big batched tails
            nc.sync.dma_start(out=w_sb[:, :], in_=w_e[:, :])
            nc.gpsimd.dma_start(out=xT[:, _sl(0)], in_=xT_e[:, _sl(0)])
            nc.scalar.dma_start(out=cosx[:, _sl(0)], in_=cosx_e[:, _sl(0)])
            nc.sync.dma_start(out=sinx[:, _sl(0)], in_=sinx_e[:, _sl(0)])
            nc.gpsimd.dma_start(out=xT[:, _sl(1)], in_=xT_e[:, _sl(1)])
            nc.scalar.dma_start(out=cosx[:, _sl(1)], in_=cosx_e[:, _sl(1)])
            nc.sync.dma_start(out=sinx[:, _sl(1)], in_=sinx_e[:, _sl(1)])
            nc.gpsimd.dma_start(out=mask01[:, :], in_=mask01_e[:, :])
            bal.gp += 3 * C_DMA_G
            rest = slice(2 * 512, T)
            nc.gpsimd.dma_start(out=xT[:, rest], in_=xT_e[:, rest])
            bal.gp += C_DMA_G
            nc.scalar.dma_start(out=cosx[:, rest], in_=cosx_e[:, rest])
            nc.sync.dma_start(out=sinx[:, rest], in_=sinx_e[:, rest])

            # HAM warm-up burns: PE busy from ~0.3us
            scratch = cpool.tile([128, 512], BF16)
            nc.vector.memset(scratch[:, :], 0.0)
            burn = ps.tile([80, 512], F32, tag="outT", bufs=2, name="burn")
            for _ in range(10):
                nc.tensor.matmul(
                    burn[:, :], scratch[:, 0:80], scratch[:, :],
                    start=True, stop=True,
                )

            q2 = cpool.tile([128, T], BF16)
            k2 = cpool.tile([128, T], BF16)

            # v tiles + mask column: [t, j_tile, 65]
            vplus = cpool.tile([128, NT, HS + 1], BF16)
            nc.vector.tensor_copy(vplus[:, :, HS], mask01[:, :])
            # fp8 interleaved-pair copy of v for DoubleRow PV (80-col padded
            # stationary so the Ko step is 16B-aligned; pad stays zero)
            vplus_f8 = cpool.tile([128, NT // 2, 2, 80], F8)
            nc.vector.memset(vplus_f8[:, :, :, :], 0.0)

            def rope_finish(dst, sl, m1, m2):
                if bal.pick2([("dve", C_ADD_D), ("gp", C_ADD_G)]) == "dve":
                    nc.vector.tensor_add(dst[:, sl], m1[:, :], m2[:, :])
                else:
                    nc.gpsimd.tensor_tensor(
                        dst[:, sl], m1[:, :], m2[:, :], ALU.add
                    )

            def proj_q(ch):
                sl = slice(ch * 512, (ch + 1) * 512)
                raw = ps.tile([128, 512], F32, tag="proj", bufs=2, name=f"qr{ch}")
                nc.tensor.matmul(raw[:, :], w_sb[:, 0:128], xT[:, sl], start=True, stop=True)
                swp = ps.tile([128, 512], F32, tag="proj", bufs=2, name=f"qs{ch}")
                nc.tensor.matmul(swp[:, :], w_sb[:, 128:256], xT[:, sl], start=True, stop=True)
                bal.dve += 2 * C_MUL_D
                m1 = wpool.tile([128, 512], BF16, tag="rope", bufs=4, name=f"m1_{ch}")
                nc.vector.tensor_mul(m1[:, :], raw[:, :], cosx[:, sl])
                m2 = wpool.tile([128, 512], BF16, tag="rope", bufs=4, name=f"m2_{ch}")
                nc.vector.tensor_mul(m2[:, :], swp[:, :], sinx[:, sl])
                rope_finish(q2, sl, m1, m2)

            def proj_k(ch):
                sl = slice(ch * 512, (ch + 1) * 512)
                raw = ps.tile([128, 512], F32, tag="proj", bufs=2, name=f"kr{ch}")
                nc.tensor.matmul(raw[:, :], w_sb[:, 256:384], xT[:, sl], start=True, stop=True)
                swp = ps.tile([128, 512], F32, tag="proj", bufs=2, name=f"ks{ch}")
                nc.tensor.matmul(swp[:, :], w_sb[:, 384:512], xT[:, sl], start=True, stop=True)
                bal.dve += 2 * C_MUL_D
                m3 = wpool.tile([128, 512], BF16, tag="rope", bufs=4, name=f"m3_{ch}")
                nc.vector.tensor_mul(m3[:, :], raw[:, :], cosx[:, sl])
                m4 = wpool.tile([128, 512], BF16, tag="rope", bufs=4, name=f"m4_{ch}")
                nc.vector.tensor_mul(m4[:, :], swp[:, :], sinx[:, sl])
                rope_finish(k2, sl, m3, m4)

            def v_block(ch):
                for tt in range(4):
                    jt = ch * 4 + tt
                    v_ps = ps.tile([128, HS], F32, tag="proj", bufs=2, name=f"v{jt}")
                    nc.tensor.matmul(
                        v_ps[:, :],
                        xT[:, jt * 128 : (jt + 1) * 128],
                        w_sb[:, 512:576],
                        start=True,
                        stop=True,
                    )
                    if bal.pick2([("act", C_V_A), ("dve", C_V_D)]) == "act":
                        nc.scalar.activation(
                            vplus[:, jt, 0:HS], v_ps[:, :], AF.Copy,
                            bias=0.0, scale=mask01[:, jt : jt + 1],
                        )
                    else:
                        nc.vector.tensor_scalar_mul(
                            vplus[:, jt, 0:HS], v_ps[:, :], mask01[:, jt : jt + 1]
                        )
                    if bal.pick2([("gp", C_VF8_G), ("dve", C_VF8_D)]) == "gp":
                        nc.gpsimd.tensor_copy(
                            vplus_f8[:, jt // 2, jt % 2, 0 : HS + 1], vplus[:, jt, :]
                        )
                    else:
                        nc.vector.tensor_copy(
                            vplus_f8[:, jt // 2, jt % 2, 0 : HS + 1], vplus[:, jt, :]
                        )

            # deep projection head: 3 chunks ahead
            proj_q(0)
            proj_k(0)
            proj_q(1)
            proj_k(1)
            v_block(0)
            proj_q(2)
            proj_k(2)
            v_block(1)

            # flat group list with background projection work attached
            work = []  # (ic, group, is_last_of_chunk, bg_blocks)
            for ic in range(NCH):
                gs = _groups_for_chunk(ic)
                for gi, g in enumerate(gs):
                    bg = []
                    if gi == 0 and ic + 3 < NCH:
                        bg.append(("pq", ic + 3))
                    if gi == min(1, len(gs) - 1) and ic + 3 < NCH:
                        bg.append(("pk", ic + 3))
                    if gi == min(2, len(gs) - 1) and 2 <= ic + 2 < NCH:
                        bg.append(("v", ic + 2))
                    work.append((ic, g, gi == len(gs) - 1, bg))

            def emit_scores(ic, g, sg):
                isl0 = ic * 512
                for jt, i_lo, off, w, _d, ro in g:
                    nc.tensor.matmul(
                        sg[:, off : off + w],
                        k2[ro : ro + HS, jt * 128 : (jt + 1) * 128],
                        q2[ro : ro + HS, isl0 + i_lo : isl0 + 512],
                        start=True,
                        stop=True,
                        tile_position=(ro, 0),
                    )

            def emit_pv(ic, g, pt, outT):
                njt = 4 * (ic + 1)
                diag = any(e[4] is not None for e in g)
                if not diag:
                    # far pair -> one fp8 DoubleRow matmul (K=256)
                    pair = g[0][0] // 2
                    nc.tensor.matmul(
                        outT[:, 0:512],
                        vplus_f8[:, pair, :, :],
                        pt[:, 0:1024].rearrange("p (r n) -> p r n", r=2),
                        start=(g[0][0] == 0),
                        stop=False,
                        perf_mode=PM.DoubleRow,
                        skip_group_check=True,
                    )
                    return
                for jt, i_lo, off, w, _d, _ro in g:
                    nc.tensor.matmul(
                        outT[0 : HS + 1, i_lo:512],
                        vplus[:, jt, :],
                        pt[:, off : off + w],
                        start=(jt == 0),
                        stop=(jt == njt - 1),
                        skip_group_check=True,
                    )

            def emit_out(pic, pouT):
                lo = 128 if pic == NCH - 1 else 0  # chunk 7 [0:128] shipped early
                osb = wpool.tile([HS + 1, 512], F32, tag="osb", bufs=2, name=f"osb{pic}")
                if bal.pick2([("act", C_OSB_A), ("dve", C_OSB_D)]) == "act":
                    nc.scalar.activation(
                        osb[:, lo:512], pouT[0 : HS + 1, lo:512], AF.Copy, bias=0.0, scale=1.0
                    )
                else:
                    nc.vector.tensor_copy(osb[:, lo:512], pouT[0 : HS + 1, lo:512])
                o0 = pic * 512
                nc.sync.dma_start(out=out_e[:, o0 + lo : o0 + 256], in_=osb[:, lo:256])
                nc.gpsimd.dma_start(out=out_e[:, o0 + 256 : o0 + 512], in_=osb[:, 256:512])
                bal.gp += C_DMA_G

            pending = None  # (ic, group, pt, outT, last?)
            outT_cur = None
            gidx = 0
            for ic, g, last, bg in work:
                diag = any(e[4] is not None for e in g)
                fd = g[-1][2] + g[-1][3]
                if g[0][0] == 0:
                    outT_cur = ps.tile(
                        [80, 512], F32, tag="outT", bufs=2, name=f"oT{ic}"
                    )
                sg = ps.tile([128, 1024], F32, tag="sg", bufs=2, name=f"sg{ic}_{g[0][0]}")
                emit_scores(ic, g, sg)
                # HAM hold: a zero-contribution accumulate matmul into the live
                # outT bank keeps the PE activity monitor fed through stalls
                # (stationary is all-zero scratch, so outT += 0; no pool churn)
                if ldw_burn:
                    nc.tensor.matmul(
                        outT_cur[:, 0:32], scratch[:, 0:80], scratch[:, 0:32],
                        start=False, stop=False, skip_group_check=True,
                    )
                gidx += 1
                if pending is not None:
                    pic, pg, ppt, pouT, plast = pending
                    emit_pv(pic, pg, ppt, pouT)
                    if plast:
                        emit_out(pic, pouT)
                    elif pic == NCH - 1 and pg[0][4] is not None:
                        osb7 = wpool.tile(
                            [HS + 1, 128], F32, tag="osb7", bufs=1, name="osb7p"
                        )
                        nc.vector.tensor_copy(osb7[:, :], pouT[0 : HS + 1, 0:128])
                        nc.sync.dma_start(
                            out=out_e[:, pic * 512 : pic * 512 + 128],
                            in_=osb7[:, :],
                        )
                # background projections first: DVE queue serves the rope
                # muls (producer-critical) before a long exp
                for kind, cc in bg:
                    if kind == "pq":
                        proj_q(cc)
                    elif kind == "pk":
                        proj_k(cc)
                    else:
                        v_block(cc)
                if diag:
                    pt = wpool.tile(
                        [128, 1024], BF16, tag="ptd", bufs=2,
                        name=f"pt{ic}_{g[0][0]}",
                    )
                    bal.act += C_GRP_A(fd)
                    nc.scalar.activation(
                        pt[:, 0:fd], sg[:, 0:fd], AF.Exp, bias=0.0, scale=SCALE
                    )
                else:
                    pt = wpool.tile(
                        [128, 1024], F8, tag="pt", bufs=4,
                        name=f"pt{ic}_{g[0][0]}",
                    )
                    eng = bal.pick2([("act", C_GRP_A(fd)), ("dve", C_GRP_D(fd))])
                    if eng == "act":
                        nc.scalar.activation(
                            pt[:, 0:fd], sg[:, 0:fd], AF.Exp, bias=0.0, scale=SCALE
                        )
                    else:
                        nc.vector.tensor_scalar(
                            pt[:, 0:fd].bitcast(I8), sg[:, 0:fd], EA8, EB8,
                            ALU.mult, ALU.add,
                        )
                for _jt, _i_lo, _off, _w, d, _ro in g:
                    if d is not None:
                        bal.gp += C_AFF_G
                        nc.gpsimd.affine_select(
                            out=pt[:, d : d + 128],
                            in_=pt[:, d : d + 128],
                            compare_op=ALU.is_ge,
                            fill=0.0,
                            base=0,
                            pattern=[[1, 128]],
                            channel_multiplier=-1,
                        )
                pending = (ic, g, pt, outT_cur, last)

            pic, pg, ppt, pouT, plast = pending
            emit_pv(pic, pg, ppt, pouT)
            emit_out(pic, pouT)

    import concourse.mybir as mybir
    _split_excess_waits(nc, mybir, limit=1)
    if os.environ.get("KERNEL_DEBUG_BAL"):
        print(f"balancer: act={bal.act/1000:.1f}us dve={bal.dve/1000:.1f}us gp={bal.gp/1000:.1f}us")
    return nc


def _get_nc():
    if "nc" not in _CACHE:
        _CACHE["nc"] = _build_nc()
    return _CACHE["nc"]


def kernel(x_text_emb, Wq, Wk, Wv, freqs_cos, freqs_sin, x_latex_mask):
    import ml_dtypes
    from concourse.bass_utils import run_bass_kernel_spmd

    bf16 = ml_dtypes.bfloat16
    nc = _get_nc()

    swap = np.arange(HS) ^ 1
    cos2 = np.repeat(np.asarray(freqs_cos, np.float32).T, 2, axis=0)  # [64, T]
    sin2s = np.repeat(np.asarray(freqs_sin, np.float32).T, 2, axis=0)
    sin2s[0::2] *= -1.0
    cosx = np.ascontiguousarray(np.tile(cos2, (2, 1))).astype(bf16)  # [128, T]
    sinx = np.ascontiguousarray(np.tile(sin2s, (2, 1))).astype(bf16)
    Wq = np.asarray(Wq, np.float32)
    Wk = np.asarray(Wk, np.float32)
    Wv = np.asarray(Wv, np.float32)
    w = np.concatenate(
        [Wq, Wq, Wq[:, swap], Wq[:, swap], Wk, Wk, Wk[:, swap], Wk[:, swap], Wv],
        axis=1,
    ).astype(bf16)
    w = np.ascontiguousarray(w)
    mask01 = np.asarray(x_latex_mask != 0, np.float32).reshape(N_CORES, NT, 128)

    in_maps = []
    for b in range(N_CORES):
        in_maps.append(
            {
                "p_xt": np.ascontiguousarray(
                    np.asarray(x_text_emb[b], np.float32).T
                ).astype(bf16),
                "p_w": w,
                "p_cos": cosx,
                "p_sin": sinx,
                "p_mask": np.ascontiguousarray(mask01[b].T),
            }
        )

    res = run_bass_kernel_spmd(nc, in_maps, core_ids=list(range(N_CORES)))
    outs = []
    for b in range(N_CORES):
        r = np.asarray(res.results[b]["p_out"], np.float32)
        outs.append((r[0:HS, :] / r[HS : HS + 1, :]).T)
    return np.stack(outs, axis=0)


# revision 12
# speedup vs baseline: 1.3186x; 1.3009x over previous
"""Single-head causal attention with RoPE + padding mask, data-parallel
over batch across 8 TRN2 NeuronCores (one batch element per core).

Per core (T=4096, C=128, HS=64):
  q = rope(x @ Wq); k = rope(x @ Wk); v = x @ Wv
  S^T[j,i] = k[j]·q[i]           (scores, transposed layout: partition=j)
  P^T = exp(S^T/sqrt(C)) * tri(i>=j)
  outT[d,i] = sum_j (mask[j]*v[j,d]) P^T[j,i]; rowsum via a mask column
        appended to v.  final out[i,d] = outT[d,i]/rowsum[i] on HOST.

Two-engine exp: ScalarE ACTIVATE handles the diagonal-band groups
(exact exp + gpsimd triangle select) plus a balanced share of the far
groups; VectorE handles the rest via a one-instruction Schraudolph
approximation: int16(bits) = rint(s*A + B) interpreted as bf16 --
|rel err| <= 3.1% per weight, harmless for rows >= 129 keys (error
averages over many keys; only diagonal-band rows are few-key).

Load balancing: rope muls are forced DVE (PSUM source); the rope adds
(SBUF-only) split DVE/GPSIMD; v-scale and output copies split ACT/DVE
(ACT runs them as activation-Copy with per-partition scale).
Projections run 3 chunks ahead of the score pipeline so the
proj->rope->scores latency never starves the PE (HAM re-throttles the
PE clock to 1.2GHz after ~3.4us of low activity -- LDWEIGHTS burns
bridge the thin early chunks).
"""

import numpy as np
import os

T, C, HS = 4096, 128, 64
N_CORES = 8
NT = T // 128      # 32 j-tiles of 128
NCH = T // 512     # 8 i-chunks of 512
SCALE = float(1.0 / np.sqrt(np.float32(C)))
# Schraudolph bf16-bits exp: bits = rint(s*EA + EB); value = bf16(bits)
EA = float(128.0 * SCALE * np.log2(np.e))
EB = float(16256.0 - 5.35)
# fp8e4m3-bits exp for the fp8 PV path: bits8 = rint(s*EA8 + EB8)
EA8 = float(8.0 * SCALE * np.log2(np.e))
EB8 = float(56.0 - 0.36)

_CACHE = {}


def _install_tile_drain_patch(tile_mod):
    """This container's walrus rejects instructions with >2 sem waits; split
    Tile's final global drain into one drain per ticked processor."""
    import bass_rust
    from concourse.vector_clock import ScopedClock

    def _patched(self, tick_clock, wait_clock):
        gc = tick_clock.global_clock
        for i in range(len(gc)):
            if gc[i] <= 0:
                continue
            v = bass_rust.VectorClock()
            v.require_at_least(i, gc[i])
            d = self.nc.sync.drain()
            wait_clock.add_sem_waits(d.ins, ScopedClock({None: v}))
        self.nc.all_engine_barrier()
        assert self.sems is not None
        popped = self.nc._tile_sem_poison_stack.pop()
        assert popped is self._sem_poison
        self.nc.clear_and_free_semaphores(list(self.sems.allocated().values()))
        self.nc.all_engine_barrier()

    tile_mod.TileContext._drain_and_barrier = _patched


def _split_excess_waits(nc, mybir, limit=1):
    """This container's walrus rejects instructions with >limit sem waits.
    Hoist excess waits onto standalone EventSemaphore instructions inserted
    just before the offending instruction on the same engine queue."""
    ctr = 0
    for f in nc.m.functions:
        for b in f.blocks:
            il = b.instructions
            out = []
            changed = False
            for ins in il:
                si = ins.sync_info
                waits = list(si.on_wait) if si and si.on_wait else []
                if len(waits) > limit:
                    changed = True
                    excess = waits[: len(waits) - limit]
                    keep = waits[len(waits) - limit :]
                    for i in range(0, len(excess), limit):
                        chunk = excess[i : i + limit]
                        ev = mybir.InstEventSemaphore(
                            name=f"I-waitsplit-{ctr}",
                            engine=ins.engine,
                            ins=[],
                            outs=[],
                            sync_info=mybir.SyncInfo(on_wait=chunk, on_update=[]),
                        )
                        ctr += 1
                        nc.register_instruction(ev)
                        out.append(ev)
                    si.on_wait = keep
                out.append(ins)
            if changed:
                b.instructions = out
    return nc


def _groups_for_chunk(ic):
    """Group layout for i-chunk ic: far full-width pairs + 2 diag groups.
    Entry: (jt, i_lo, sg_off, width, diag_off, row_group)."""
    groups = []
    for p in range(0, 4 * ic, 2):
        groups.append(
            [(p, 0, 0, 512, None, 0), (p + 1, 0, 512, 512, None, 64)]
        )
    b = 4 * ic
    groups.append([(b + 0, 0, 0, 512, 0, 0), (b + 2, 256, 512, 256, 512, 64)])
    groups.append([(b + 1, 128, 0, 384, 0, 0), (b + 3, 384, 384, 128, 384, 0)])
    return groups


class _Balancer:
    """Greedy multi-queue load balancer (estimated ns per op)."""

    def __init__(self):
        self.act = 0.0
        self.dve = 0.0
        self.gp = 0.0

    def pick2(self, attr_costs):
        # attr_costs: list of (attr, cost); choose min completion time
        best = min(attr_costs, key=lambda ac: getattr(self, ac[0]) + ac[1])
        setattr(self, best[0], getattr(self, best[0]) + best[1])
        return best[0]


def _build_nc():
    import concourse.bass as bass
    import concourse.mybir as mybir
    from concourse import tile

    _install_tile_drain_patch(tile)

    DT = mybir.dt
    F32, BF16, I16 = DT.float32, DT.bfloat16, DT.int16
    I8, F8 = DT.int8, DT.float8e4
    PM = mybir.MatmulPerfMode
    AF = mybir.ActivationFunctionType
    ALU = mybir.AluOpType

    nc = bass.Bass()
    xT_e = nc.declare_dram_parameter("p_xt", [C, T], BF16, isOutput=False)
    w_e = nc.declare_dram_parameter("p_w", [C, 576], BF16, isOutput=False)
    cosx_e = nc.declare_dram_parameter("p_cos", [128, T], BF16, isOutput=False)
    sinx_e = nc.declare_dram_parameter("p_sin", [128, T], BF16, isOutput=False)
    mask01_e = nc.declare_dram_parameter("p_mask", [128, NT], F32, isOutput=False)
    out_e = nc.declare_dram_parameter("p_out", [HS + 1, T], F32, isOutput=True)

    bal = _Balancer()
    C_GRP_A = lambda fd: (fd + 352) / 1.2 + 110
    C_GRP_D = lambda fd: (fd + 120) / 0.96 + 170
    C_V_A = (64 + 352) / 1.2 + 110
    C_V_D = (64 + 120) / 0.96 + 80
    C_OSB_A = (512 + 352) / 1.2 + 110
    C_OSB_D = (512 + 120) / 0.96 + 80
    C_MUL_D = 685 + 60
    C_ADD_D = 424 + 80
    C_ADD_G = 1109 + 200
    C_AFF_G = 400.0
    C_VF8_G = 220.0
    C_VF8_D = 174.0
    C_DMA_G = 780.0
    ldw_burn = not os.environ.get("KERNEL_NO_LDWBURN")

    with tile.TileContext(nc) as tc:
        with (
            tc.tile_pool(name="const", bufs=1) as cpool,
            tc.tile_pool(name="work", bufs=3) as wpool,
            tc.tile_pool(name="ps", bufs=2, space="PSUM") as ps,
        ):
            xT = cpool.tile([C, T], BF16)
            w_sb = cpool.tile([C, 576], BF16)
            mask01 = cpool.tile([128, NT], F32)
            cosx = cpool.tile([128, T], BF16)
            sinx = cpool.tile([128, T], BF16)

            # preload the exp activation table while input DMAs run
            warm = cpool.tile([128, 8], F32)
            nc.vector.memset(warm[:, :], 0.0)
            nc.scalar.activation(warm[:, :], warm[:, :], AF.Exp, bias=0.0, scale=1.0)

            def _sl(ch):
                return slice(ch * 512, (ch + 1) * 512)

            # head-critical input DMAs, one per queue, then big batched tails
            nc.sync.dma_start(out=w_sb[:, :], in_=w_e[:, :])
            nc.gpsimd.dma_start(out=xT[:, _sl(0)], in_=xT_e[:, _sl(0)])
            nc.scalar.dma_start(out=cosx[:, _sl(0)], in_=cosx_e[:, _sl(0)])
            nc.sync.dma_start(out=sinx[:, _sl(0)], in_=sinx_e[:, _sl(0)])
            nc.gpsimd.dma_start(out=xT[:, _sl(1)], in_=xT_e[:, _sl(1)])
            nc.scalar.dma_start(out=cosx[:, _sl(1)], in_=cosx_e[:, _sl(1)])
            nc.sync.dma_start(out=sinx[:, _sl(1)], in_=sinx_e[:, _sl(1)])
            nc.gpsimd.dma_start(out=mask01[:, :], in_=mask01_e[:, :])
            bal.gp += 3 * C_DMA_G
            rest = slice(2 * 512, T)
            nc.gpsimd.dma_start(out=xT[:, rest], in_=xT_e[:, rest])
            bal.gp += C_DMA_G
            nc.scalar.dma_start(out=cosx[:, rest], in_=cosx_e[:, rest])
            nc.sync.dma_start(out=sinx[:, rest], in_=sinx_e[:, rest])

            # HAM warm-up burns: PE busy from ~0.3us
            scratch = cpool.tile([128, 512], BF16)
            nc.vector.memset(scratch[:, :], 0.0)
            burn = ps.tile([80, 512], F32, tag="outT", bufs=2, name="burn")
            for _ in range(10):
                nc.tensor.matmul(
                    burn[:, :], scratch[:, 0:80], scratch[:, :],
                    start=True, stop=True,
                )

            q2 = cpool.tile([128, T], BF16)
            k2 = cpool.tile([128, T], BF16)

            # v tiles + mask column: [t, j_tile, 65]
            vplus = cpool.tile([128, NT, HS + 1], BF16)
            nc.vector.tensor_copy(vplus[:, :, HS], mask01[:, :])
            # fp8 interleaved-pair copy of v for DoubleRow PV (80-col padded
            # stationary so the Ko step is 16B-aligned; pad stays zero)
            vplus_f8 = cpool.tile([128, NT // 2, 2, 80], F8)
            nc.vector.memset(vplus_f8[:, :, :, :], 0.0)

            def rope_finish(dst, sl, m1, m2):
                if bal.pick2([("dve", C_ADD_D), ("gp", C_ADD_G)]) == "dve":
                    nc.vector.tensor_add(dst[:, sl], m1[:, :], m2[:, :])
                else:
                    nc.gpsimd.tensor_tensor(
                        dst[:, sl], m1[:, :], m2[:, :], ALU.add
                    )

            def proj_q(ch):
                sl = slice(ch * 512, (ch + 1) * 512)
                raw = ps.tile([128, 512], F32, tag="proj", bufs=2, name=f"qr{ch}")
                nc.tensor.matmul(raw[:, :], w_sb[:, 0:128], xT[:, sl], start=True, stop=True)
                swp = ps.tile([128, 512], F32, tag="proj", bufs=2, name=f"qs{ch}")
                nc.tensor.matmul(swp[:, :], w_sb[:, 128:256], xT[:, sl], start=True, stop=True)
                bal.dve += 2 * C_MUL_D
                m1 = wpool.tile([128, 512], BF16, tag="rope", bufs=4, name=f"m1_{ch}")
                nc.vector.tensor_mul(m1[:, :], raw[:, :], cosx[:, sl])
                m2 = wpool.tile([128, 512], BF16, tag="rope", bufs=4, name=f"m2_{ch}")
                nc.vector.tensor_mul(m2[:, :], swp[:, :], sinx[:, sl])
                rope_finish(q2, sl, m1, m2)

            def proj_k(ch):
                sl = slice(ch * 512, (ch + 1) * 512)
                raw = ps.tile([128, 512], F32, tag="proj", bufs=2, name=f"kr{ch}")
                nc.tensor.matmul(raw[:, :], w_sb[:, 256:384], xT[:, sl], start=True, stop=True)
                swp = ps.tile([128, 512], F32, tag="proj", bufs=2, name=f"ks{ch}")
                nc.tensor.matmul(swp[:, :], w_sb[:, 384:512], xT[:, sl], start=True, stop=True)
                bal.dve += 2 * C_MUL_D
                m3 = wpool.tile([128, 512], BF16, tag="rope", bufs=4, name=f"m3_{ch}")
                nc.vector.tensor_mul(m3[:, :], raw[:, :], cosx[:, sl])
                m4 = wpool.tile([128, 512], BF16, tag="rope", bufs=4, name=f"m4_{ch}")
                nc.vector.tensor_mul(m4[:, :], swp[:, :], sinx[:, sl])
                rope_finish(k2, sl, m3, m4)

            def v_block(ch):
                for tt in range(4):
                    jt = ch * 4 + tt
                    v_ps = ps.tile([128, HS], F32, tag="proj", bufs=2, name=f"v{jt}")
                    nc.tensor.matmul(
                        v_ps[:, :],
                        xT[:, jt * 128 : (jt + 1) * 128],
                        w_sb[:, 512:576],
                        start=True,
                        stop=True,
                    )
                    if bal.pick2([("act", C_V_A), ("dve", C_V_D)]) == "act":
                        nc.scalar.activation(
                            vplus[:, jt, 0:HS], v_ps[:, :], AF.Copy,
                            bias=0.0, scale=mask01[:, jt : jt + 1],
                        )
                    else:
                        nc.vector.tensor_scalar_mul(
                            vplus[:, jt, 0:HS], v_ps[:, :], mask01[:, jt : jt + 1]
                        )
                    if bal.pick2([("gp", C_VF8_G), ("dve", C_VF8_D)]) == "gp":
                        nc.gpsimd.tensor_copy(
                            vplus_f8[:, jt // 2, jt % 2, 0 : HS + 1], vplus[:, jt, :]
                        )
                    else:
                        nc.vector.tensor_copy(
                            vplus_f8[:, jt // 2, jt % 2, 0 : HS + 1], vplus[:, jt, :]
                        )

            # deep projection head: 3 chunks ahead
            proj_q(0)
            proj_k(0)
            proj_q(1)
            proj_k(1)
            v_block(0)
            proj_q(2)
            proj_k(2)
            v_block(1)

            # flat group list with background projection work attached
            work = []  # (ic, group, is_last_of_chunk, bg_blocks)
            for ic in range(NCH):
                gs = _groups_for_chunk(ic)
                for gi, g in enumerate(gs):
                    bg = []
                    if gi == 0 and ic + 3 < NCH:
                        bg.append(("pq", ic + 3))
                    if gi == min(1, len(gs) - 1) and ic + 3 < NCH:
                        bg.append(("pk", ic + 3))
                    if gi == min(2, len(gs) - 1) and 2 <= ic + 2 < NCH:
                        bg.append(("v", ic + 2))
                    work.append((ic, g, gi == len(gs) - 1, bg))

            def emit_scores(ic, g, sg):
                isl0 = ic * 512
                for jt, i_lo, off, w, _d, ro in g:
                    nc.tensor.matmul(
                        sg[:, off : off + w],
                        k2[ro : ro + HS, jt * 128 : (jt + 1) * 128],
                        q2[ro : ro + HS, isl0 + i_lo : isl0 + 512],
                        start=True,
                        stop=True,
                        tile_position=(ro, 0),
                    )

            def emit_pv(ic, g, pt, outT):
                njt = 4 * (ic + 1)
                diag = any(e[4] is not None for e in g)
                if not diag:
                    # far pair -> one fp8 DoubleRow matmul (K=256)
                    pair = g[0][0] // 2
                    nc.tensor.matmul(
                        outT[:, 0:512],
                        vplus_f8[:, pair, :, :],
                        pt[:, 0:1024].rearrange("p (r n) -> p r n", r=2),
                        start=(g[0][0] == 0),
                        stop=False,
                        perf_mode=PM.DoubleRow,
                        skip_group_check=True,
                    )
                    return
                for jt, i_lo, off, w, _d, _ro in g:
                    nc.tensor.matmul(
                        outT[0 : HS + 1, i_lo:512],
                        vplus[:, jt, :],
                        pt[:, off : off + w],
                        start=(jt == 0),
                        stop=(jt == njt - 1),
                        skip_group_check=True,
                    )

            def emit_out(pic, pouT):
                lo = 128 if pic == NCH - 1 else 0  # chunk 7 [0:128] shipped early
                osb = wpool.tile([HS + 1, 512], F32, tag="osb", bufs=2, name=f"osb{pic}")
                if bal.pick2([("act", C_OSB_A), ("dve", C_OSB_D)]) == "act":
                    nc.scalar.activation(
                        osb[:, lo:512], pouT[0 : HS + 1, lo:512], AF.Copy, bias=0.0, scale=1.0
                    )
                else:
                    nc.vector.tensor_copy(osb[:, lo:512], pouT[0 : HS + 1, lo:512])
                o0 = pic * 512
                nc.sync.dma_start(out=out_e[:, o0 + lo : o0 + 256], in_=osb[:, lo:256])
                nc.gpsimd.dma_start(out=out_e[:, o0 + 256 : o0 + 512], in_=osb[:, 256:512])
                bal.gp += C_DMA_G

            def flush_pending(ent):
                pic, pg, ppt, pouT, plast = ent
                emit_pv(pic, pg, ppt, pouT)
                if plast:
                    emit_out(pic, pouT)
                elif pic == NCH - 1 and pg[0][4] is not None:
                    osb7 = wpool.tile(
                        [HS + 1, 128], F32, tag="osb7", bufs=1, name="osb7p"
                    )
                    nc.vector.tensor_copy(osb7[:, :], pouT[0 : HS + 1, 0:128])
                    nc.sync.dma_start(
                        out=out_e[:, pic * 512 : pic * 512 + 128],
                        in_=osb7[:, :],
                    )

            # software pipeline depth 2: PV of group g is emitted two slots
            # later, so its exp is long finished and the PE queue never
            # blocks on an exp engine mid-stream
            pendq = []
            outT_cur = None
            for ic, g, last, bg in work:
                diag = any(e[4] is not None for e in g)
                fd = g[-1][2] + g[-1][3]
                if g[0][0] == 0:
                    outT_cur = ps.tile(
                        [80, 512], F32, tag="outT", bufs=2, name=f"oT{ic}"
                    )
                sg = ps.tile([128, 1024], F32, tag="sg", bufs=2, name=f"sg{ic}_{g[0][0]}")
                emit_scores(ic, g, sg)
                if len(pendq) >= 2:
                    flush_pending(pendq.pop(0))
                # exp first on its queue (latency-critical), then background
                if diag:
                    pt = wpool.tile(
                        [128, 1024], BF16, tag="ptd", bufs=3,
                        name=f"pt{ic}_{g[0][0]}",
                    )
                    bal.act += C_GRP_A(fd)
                    nc.scalar.activation(
                        pt[:, 0:fd], sg[:, 0:fd], AF.Exp, bias=0.0, scale=SCALE
                    )
                else:
                    pt = wpool.tile(
                        [128, 1024], F8, tag="pt", bufs=5,
                        name=f"pt{ic}_{g[0][0]}",
                    )
                    eng = bal.pick2([("act", C_GRP_A(fd)), ("dve", C_GRP_D(fd))])
                    if eng == "act":
                        nc.scalar.activation(
                            pt[:, 0:fd], sg[:, 0:fd], AF.Exp, bias=0.0, scale=SCALE
                        )
                    else:
                        nc.vector.tensor_scalar(
                            pt[:, 0:fd].bitcast(I8), sg[:, 0:fd], EA8, EB8,
                            ALU.mult, ALU.add,
                        )
                for _jt, _i_lo, _off, _w, d, _ro in g:
                    if d is not None:
                        bal.gp += C_AFF_G
                        nc.gpsimd.affine_select(
                            out=pt[:, d : d + 128],
                            in_=pt[:, d : d + 128],
                            compare_op=ALU.is_ge,
                            fill=0.0,
                            base=0,
                            pattern=[[1, 128]],
                            channel_multiplier=-1,
                        )
                for kind, cc in bg:
                    if kind == "pq":
                        proj_q(cc)
                    elif kind == "pk":
                        proj_k(cc)
                    else:
                        v_block(cc)
                pendq.append((ic, g, pt, outT_cur, last))

            for ent in pendq:
                flush_pending(ent)

    import concourse.mybir as mybir
    _split_excess_waits(nc, mybir, limit=1)
    if os.environ.get("KERNEL_DEBUG_BAL"):
        print(f"balancer: act={bal.act/1000:.1f}us dve={bal.dve/1000:.1f}us gp={bal.gp/1000:.1f}us")
    return nc


def _get_nc():
    if "nc" not in _CACHE:
        _CACHE["nc"] = _build_nc()
    return _CACHE["nc"]


def kernel(x_text_emb, Wq, Wk, Wv, freqs_cos, freqs_sin, x_latex_mask):
    import ml_dtypes
    from concourse.bass_utils import run_bass_kernel_spmd

    bf16 = ml_dtypes.bfloat16
    nc = _get_nc()

    swap = np.arange(HS) ^ 1
    cos2 = np.repeat(np.asarray(freqs_cos, np.float32).T, 2, axis=0)  # [64, T]
    sin2s = np.repeat(np.asarray(freqs_sin, np.float32).T, 2, axis=0)
    sin2s[0::2] *= -1.0
    cosx = np.ascontiguousarray(np.tile(cos2, (2, 1))).astype(bf16)  # [128, T]
    sinx = np.ascontiguousarray(np.tile(sin2s, (2, 1))).astype(bf16)
    Wq = np.asarray(Wq, np.float32)
    Wk = np.asarray(Wk, np.float32)
    Wv = np.asarray(Wv, np.float32)
    w = np.concatenate(
        [Wq, Wq, Wq[:, swap], Wq[:, swap], Wk, Wk, Wk[:, swap], Wk[:, swap], Wv],
        axis=1,
    ).astype(bf16)
    w = np.ascontiguousarray(w)
    mask01 = np.asarray(x_latex_mask != 0, np.float32).reshape(N_CORES, NT, 128)

    in_maps = []
    for b in range(N_CORES):
        in_maps.append(
            {
                "p_xt": np.ascontiguousarray(
                    np.asarray(x_text_emb[b], np.float32).T
                ).astype(bf16),
                "p_w": w,
                "p_cos": cosx,
                "p_sin": sinx,
                "p_mask": np.ascontiguousarray(mask01[b].T),
            }
        )

    res = run_bass_kernel_spmd(nc, in_maps, core_ids=list(range(N_CORES)))
    outs = []
    for b in range(N_CORES):
        r = np.asarray(res.results[b]["p_out"], np.float32)
        outs.append((r[0:HS, :] / r[HS : HS + 1, :]).T)
    return np.stack(outs, axis=0)


# revision 14
# speedup vs baseline: 1.3226x; 1.0030x over previous
"""Single-head causal attention with RoPE + padding mask, data-parallel
over batch across 8 TRN2 NeuronCores (one batch element per core).

Per core (T=4096, C=128, HS=64):
  q = rope(x @ Wq); k = rope(x @ Wk); v = x @ Wv
  S^T[j,i] = k[j]·q[i]           (scores, transposed layout: partition=j)
  P^T = exp(S^T/sqrt(C)) * tri(i>=j)
  outT[d,i] = sum_j (mask[j]*v[j,d]) P^T[j,i]; rowsum via a mask column
        appended to v.  final out[i,d] = outT[d,i]/rowsum[i] on HOST.

Two-engine exp: ScalarE ACTIVATE handles the diagonal-band groups
(exact exp + gpsimd triangle select) plus a balanced share of the far
groups; VectorE handles the rest via a one-instruction Schraudolph
approximation: int16(bits) = rint(s*A + B) interpreted as bf16 --
|rel err| <= 3.1% per weight, harmless for rows >= 129 keys (error
averages over many keys; only diagonal-band rows are few-key).

Load balancing: rope muls are forced DVE (PSUM source); the rope adds
(SBUF-only) split DVE/GPSIMD; v-scale and output copies split ACT/DVE
(ACT runs them as activation-Copy with per-partition scale).
Projections run 3 chunks ahead of the score pipeline so the
proj->rope->scores latency never starves the PE (HAM re-throttles the
PE clock to 1.2GHz after ~3.4us of low activity -- LDWEIGHTS burns
bridge the thin early chunks).
"""

import numpy as np
import os

T, C, HS = 4096, 128, 64
N_CORES = 8
NT = T // 128      # 32 j-tiles of 128
NCH = T // 512     # 8 i-chunks of 512
SCALE = float(1.0 / np.sqrt(np.float32(C)))
# Schraudolph bf16-bits exp: bits = rint(s*EA + EB); value = bf16(bits)
EA = float(128.0 * SCALE * np.log2(np.e))
EB = float(16256.0 - 5.35)
# fp8e4m3-bits exp for the fp8 PV path: bits8 = rint(s*EA8 + EB8)
EA8 = float(8.0 * SCALE * np.log2(np.e))
EB8 = float(56.0 - 0.36)

_CACHE = {}


def _install_tile_drain_patch(tile_mod):
    """This container's walrus rejects instructions with >2 sem waits; split
    Tile's final global drain into one drain per ticked processor."""
    import bass_rust
    from concourse.vector_clock import ScopedClock

    def _patched(self, tick_clock, wait_clock):
        gc = tick_clock.global_clock
        for i in range(len(gc)):
            if gc[i] <= 0:
                continue
            v = bass_rust.VectorClock()
            v.require_at_least(i, gc[i])
            d = self.nc.sync.drain()
            wait_clock.add_sem_waits(d.ins, ScopedClock({None: v}))
        self.nc.all_engine_barrier()
        assert self.sems is not None
        popped = self.nc._tile_sem_poison_stack.pop()
        assert popped is self._sem_poison
        self.nc.clear_and_free_semaphores(list(self.sems.allocated().values()))
        self.nc.all_engine_barrier()

    tile_mod.TileContext._drain_and_barrier = _patched


def _split_excess_waits(nc, mybir, limit=1):
    """This container's walrus rejects instructions with >limit sem waits.
    Hoist excess waits onto standalone EventSemaphore instructions inserted
    just before the offending instruction on the same engine queue."""
    ctr = 0
    for f in nc.m.functions:
        for b in f.blocks:
            il = b.instructions
            out = []
            changed = False
            for ins in il:
                si = ins.sync_info
                waits = list(si.on_wait) if si and si.on_wait else []
                if len(waits) > limit:
                    changed = True
                    excess = waits[: len(waits) - limit]
                    keep = waits[len(waits) - limit :]
                    for i in range(0, len(excess), limit):
                        chunk = excess[i : i + limit]
                        ev = mybir.InstEventSemaphore(
                            name=f"I-waitsplit-{ctr}",
                            engine=ins.engine,
                            ins=[],
                            outs=[],
                            sync_info=mybir.SyncInfo(on_wait=chunk, on_update=[]),
                        )
                        ctr += 1
                        nc.register_instruction(ev)
                        out.append(ev)
                    si.on_wait = keep
                out.append(ins)
            if changed:
                b.instructions = out
    return nc


def _groups_for_chunk(ic):
    """Group layout for i-chunk ic: far full-width pairs + 2 diag groups.
    Entry: (jt, i_lo, sg_off, width, diag_off, row_group)."""
    groups = []
    for p in range(0, 4 * ic, 2):
        groups.append(
            [(p, 0, 0, 512, None, 0), (p + 1, 0, 512, 512, None, 64)]
        )
    b = 4 * ic
    groups.append([(b + 0, 0, 0, 512, 0, 0), (b + 2, 256, 512, 256, 512, 64)])
    groups.append([(b + 1, 128, 0, 384, 0, 0), (b + 3, 384, 384, 128, 384, 0)])
    return groups


class _Balancer:
    """Greedy multi-queue load balancer (estimated ns per op)."""

    def __init__(self):
        self.act = 0.0
        self.dve = 0.0
        self.gp = 0.0

    def pick2(self, attr_costs):
        # attr_costs: list of (attr, cost); choose min completion time
        best = min(attr_costs, key=lambda ac: getattr(self, ac[0]) + ac[1])
        setattr(self, best[0], getattr(self, best[0]) + best[1])
        return best[0]


def _build_nc():
    import concourse.bass as bass
    import concourse.mybir as mybir
    from concourse import tile

    _install_tile_drain_patch(tile)

    DT = mybir.dt
    F32, BF16, I16 = DT.float32, DT.bfloat16, DT.int16
    I8, F8 = DT.int8, DT.float8e4
    PM = mybir.MatmulPerfMode
    AF = mybir.ActivationFunctionType
    ALU = mybir.AluOpType

    nc = bass.Bass()
    xT_e = nc.declare_dram_parameter("p_xt", [C, T], BF16, isOutput=False)
    w_e = nc.declare_dram_parameter("p_w", [C, 576], BF16, isOutput=False)
    cosx_e = nc.declare_dram_parameter("p_cos", [128, T], BF16, isOutput=False)
    sinx_e = nc.declare_dram_parameter("p_sin", [128, T], BF16, isOutput=False)
    mask01_e = nc.declare_dram_parameter("p_mask", [128, NT], F32, isOutput=False)
    out_e = nc.declare_dram_parameter("p_out", [HS + 1, T], F32, isOutput=True)

    bal = _Balancer()
    C_GRP_A = lambda fd: (fd + 352) / 1.2 + 110
    C_GRP_D = lambda fd: (fd + 120) / 0.96 + 170
    C_V_A = (64 + 352) / 1.2 + 110
    C_V_D = (64 + 120) / 0.96 + 80
    C_OSB_A = (512 + 352) / 1.2 + 110
    C_OSB_D = (512 + 120) / 0.96 + 80
    C_MUL_D = 685 + 60
    C_ADD_D = 424 + 80
    C_ADD_G = 1109 + 200
    C_AFF_G = 400.0
    C_VF8_G = 220.0
    C_VF8_D = 174.0
    C_DMA_G = 780.0
    ldw_burn = not os.environ.get("KERNEL_NO_LDWBURN")

    with tile.TileContext(nc) as tc:
        with (
            tc.tile_pool(name="const", bufs=1) as cpool,
            tc.tile_pool(name="work", bufs=3) as wpool,
            tc.tile_pool(name="ps", bufs=2, space="PSUM") as ps,
        ):
            xT = cpool.tile([C, T], BF16)
            w_sb = cpool.tile([C, 576], BF16)
            mask01 = cpool.tile([128, NT], F32)
            cosx = cpool.tile([128, T], BF16)
            sinx = cpool.tile([128, T], BF16)

            # preload the exp activation table while input DMAs run
            warm = cpool.tile([128, 8], F32)
            nc.vector.memset(warm[:, :], 0.0)
            nc.scalar.activation(warm[:, :], warm[:, :], AF.Exp, bias=0.0, scale=1.0)

            def _sl(ch):
                return slice(ch * 512, (ch + 1) * 512)

            # per-chunk input DMAs: x on the gpsimd queue, cos/sin/w on the
            # sync queue, NOTHING on the scalar queue (ACT must start exp asap)
            nc.sync.dma_start(out=w_sb[:, :], in_=w_e[:, :])
            nc.gpsimd.dma_start(out=xT[:, _sl(0)], in_=xT_e[:, _sl(0)])
            nc.sync.dma_start(out=cosx[:, _sl(0)], in_=cosx_e[:, _sl(0)])
            nc.sync.dma_start(out=sinx[:, _sl(0)], in_=sinx_e[:, _sl(0)])
            nc.gpsimd.dma_start(out=xT[:, _sl(1)], in_=xT_e[:, _sl(1)])
            nc.gpsimd.dma_start(out=mask01[:, :], in_=mask01_e[:, :])
            bal.gp += 3 * C_DMA_G
            for ch in range(2, NCH):
                nc.gpsimd.dma_start(out=xT[:, _sl(ch)], in_=xT_e[:, _sl(ch)])
                bal.gp += C_DMA_G
            for ch in range(1, NCH):
                nc.sync.dma_start(out=cosx[:, _sl(ch)], in_=cosx_e[:, _sl(ch)])
                nc.sync.dma_start(out=sinx[:, _sl(ch)], in_=sinx_e[:, _sl(ch)])

            scratch = cpool.tile([128, 1024], BF16)
            nc.vector.memset(scratch[:, :], 0.0)
            burn = ps.tile([80, 512], F32, tag="outT", bufs=2, name="burn")

            q2 = cpool.tile([128, T], BF16)
            k2 = cpool.tile([128, T], BF16)

            # v tiles + mask column: [t, j_tile, 65]
            vplus = cpool.tile([128, NT, HS + 1], BF16)
            nc.vector.tensor_copy(vplus[:, :, HS], mask01[:, :])
            # fp8 interleaved-pair copy of v for DoubleRow PV (80-col padded
            # stationary so the Ko step is 16B-aligned; pad stays zero)
            vplus_f8 = cpool.tile([128, NT // 2, 2, 80], F8)
            nc.vector.memset(vplus_f8[:, :, :, :], 0.0)

            def rope_finish(dst, sl, m1, m2):
                if bal.pick2([("dve", C_ADD_D), ("gp", C_ADD_G)]) == "dve":
                    nc.vector.tensor_add(dst[:, sl], m1[:, :], m2[:, :])
                else:
                    nc.gpsimd.tensor_tensor(
                        dst[:, sl], m1[:, :], m2[:, :], ALU.add
                    )

            def proj_q(ch):
                sl = slice(ch * 512, (ch + 1) * 512)
                raw = ps.tile([128, 512], F32, tag="proj", bufs=2, name=f"qr{ch}")
                nc.tensor.matmul(raw[:, :], w_sb[:, 0:128], xT[:, sl], start=True, stop=True)
                swp = ps.tile([128, 512], F32, tag="proj", bufs=2, name=f"qs{ch}")
                nc.tensor.matmul(swp[:, :], w_sb[:, 128:256], xT[:, sl], start=True, stop=True)
                bal.dve += 2 * C_MUL_D
                m1 = wpool.tile([128, 512], BF16, tag="rope", bufs=4, name=f"m1_{ch}")
                nc.vector.tensor_mul(m1[:, :], raw[:, :], cosx[:, sl])
                m2 = wpool.tile([128, 512], BF16, tag="rope", bufs=4, name=f"m2_{ch}")
                nc.vector.tensor_mul(m2[:, :], swp[:, :], sinx[:, sl])
                rope_finish(q2, sl, m1, m2)

            def proj_k(ch):
                sl = slice(ch * 512, (ch + 1) * 512)
                raw = ps.tile([128, 512], F32, tag="proj", bufs=2, name=f"kr{ch}")
                nc.tensor.matmul(raw[:, :], w_sb[:, 256:384], xT[:, sl], start=True, stop=True)
                swp = ps.tile([128, 512], F32, tag="proj", bufs=2, name=f"ks{ch}")
                nc.tensor.matmul(swp[:, :], w_sb[:, 384:512], xT[:, sl], start=True, stop=True)
                bal.dve += 2 * C_MUL_D
                m3 = wpool.tile([128, 512], BF16, tag="rope", bufs=4, name=f"m3_{ch}")
                nc.vector.tensor_mul(m3[:, :], raw[:, :], cosx[:, sl])
                m4 = wpool.tile([128, 512], BF16, tag="rope", bufs=4, name=f"m4_{ch}")
                nc.vector.tensor_mul(m4[:, :], swp[:, :], sinx[:, sl])
                rope_finish(k2, sl, m3, m4)

            def v_block(ch):
                for tt in range(4):
                    jt = ch * 4 + tt
                    v_ps = ps.tile([128, HS], F32, tag="proj", bufs=2, name=f"v{jt}")
                    nc.tensor.matmul(
                        v_ps[:, :],
                        xT[:, jt * 128 : (jt + 1) * 128],
                        w_sb[:, 512:576],
                        start=True,
                        stop=True,
                    )
                    if bal.pick2([("act", C_V_A), ("dve", C_V_D)]) == "act":
                        nc.scalar.activation(
                            vplus[:, jt, 0:HS], v_ps[:, :], AF.Copy,
                            bias=0.0, scale=mask01[:, jt : jt + 1],
                        )
                    else:
                        nc.vector.tensor_scalar_mul(
                            vplus[:, jt, 0:HS], v_ps[:, :], mask01[:, jt : jt + 1]
                        )
                    if bal.pick2([("gp", C_VF8_G), ("dve", C_VF8_D)]) == "gp":
                        nc.gpsimd.tensor_copy(
                            vplus_f8[:, jt // 2, jt % 2, 0 : HS + 1], vplus[:, jt, :]
                        )
                    else:
                        nc.vector.tensor_copy(
                            vplus_f8[:, jt // 2, jt % 2, 0 : HS + 1], vplus[:, jt, :]
                        )

            # deep projection head: 3 chunks ahead, then HAM warm-up burns
            # bridge the thin early chunks at full PE-queue density
            proj_q(0)
            proj_k(0)
            proj_q(1)
            proj_k(1)
            v_block(0)
            proj_q(2)
            proj_k(2)
            v_block(1)
            for _ in range(11):
                nc.tensor.matmul(
                    burn[:, 0:512], scratch[:, 0:80], scratch[:, 0:512],
                    start=True, stop=True,
                )

            # flat group list with background projection work attached
            work = []  # (ic, group, is_last_of_chunk, bg_blocks)
            for ic in range(NCH):
                gs = _groups_for_chunk(ic)
                for gi, g in enumerate(gs):
                    bg = []
                    if gi == 0 and ic + 3 < NCH:
                        bg.append(("pq", ic + 3))
                    if gi == min(1, len(gs) - 1) and ic + 3 < NCH:
                        bg.append(("pk", ic + 3))
                    if gi == min(2, len(gs) - 1) and 2 <= ic + 2 < NCH:
                        bg.append(("v", ic + 2))
                    work.append((ic, g, gi == len(gs) - 1, bg))

            def emit_scores(ic, g, sg):
                isl0 = ic * 512
                for jt, i_lo, off, w, _d, ro in g:
                    nc.tensor.matmul(
                        sg[:, off : off + w],
                        k2[ro : ro + HS, jt * 128 : (jt + 1) * 128],
                        q2[ro : ro + HS, isl0 + i_lo : isl0 + 512],
                        start=True,
                        stop=True,
                        tile_position=(ro, 0),
                    )

            def emit_pv(ic, g, pt, outT):
                njt = 4 * (ic + 1)
                diag = any(e[4] is not None for e in g)
                if not diag:
                    # far pair -> one fp8 DoubleRow matmul (K=256)
                    pair = g[0][0] // 2
                    nc.tensor.matmul(
                        outT[:, 0:512],
                        vplus_f8[:, pair, :, :],
                        pt[:, 0:1024].rearrange("p (r n) -> p r n", r=2),
                        start=(g[0][0] == 0),
                        stop=False,
                        perf_mode=PM.DoubleRow,
                        skip_group_check=True,
                    )
                    return
                for jt, i_lo, off, w, _d, _ro in g:
                    nc.tensor.matmul(
                        outT[0 : HS + 1, i_lo:512],
                        vplus[:, jt, :],
                        pt[:, off : off + w],
                        start=(jt == 0),
                        stop=(jt == njt - 1),
                        skip_group_check=True,
                    )

            def emit_out(pic, pouT):
                lo = 128 if pic == NCH - 1 else 0  # chunk 7 [0:128] shipped early
                osb = wpool.tile([HS + 1, 512], F32, tag="osb", bufs=2, name=f"osb{pic}")
                if bal.pick2([("act", C_OSB_A), ("dve", C_OSB_D)]) == "act":
                    nc.scalar.activation(
                        osb[:, lo:512], pouT[0 : HS + 1, lo:512], AF.Copy, bias=0.0, scale=1.0
                    )
                else:
                    nc.vector.tensor_copy(osb[:, lo:512], pouT[0 : HS + 1, lo:512])
                o0 = pic * 512
                nc.sync.dma_start(out=out_e[:, o0 + lo : o0 + 256], in_=osb[:, lo:256])
                nc.gpsimd.dma_start(out=out_e[:, o0 + 256 : o0 + 512], in_=osb[:, 256:512])
                bal.gp += C_DMA_G

            def flush_pending(ent):
                pic, pg, ppt, pouT, plast = ent
                emit_pv(pic, pg, ppt, pouT)
                if plast:
                    emit_out(pic, pouT)
                elif pic == NCH - 1 and pg[0][4] is not None:
                    osb7 = wpool.tile(
                        [HS + 1, 128], F32, tag="osb7", bufs=1, name="osb7p"
                    )
                    nc.vector.tensor_copy(osb7[:, :], pouT[0 : HS + 1, 0:128])
                    nc.sync.dma_start(
                        out=out_e[:, pic * 512 : pic * 512 + 128],
                        in_=osb7[:, :],
                    )

            # software pipeline depth 2: PV of group g is emitted two slots
            # later, so its exp is long finished and the PE queue never
            # blocks on an exp engine mid-stream
            pendq = []
            outT_cur = None
            for ic, g, last, bg in work:
                diag = any(e[4] is not None for e in g)
                fd = g[-1][2] + g[-1][3]
                if g[0][0] == 0:
                    outT_cur = ps.tile(
                        [80, 512], F32, tag="outT", bufs=2, name=f"oT{ic}"
                    )
                sg = ps.tile([128, 1024], F32, tag="sg", bufs=2, name=f"sg{ic}_{g[0][0]}")
                emit_scores(ic, g, sg)
                if len(pendq) >= 2:
                    flush_pending(pendq.pop(0))
                # exp first on its queue (latency-critical), then background
                if diag:
                    pt = wpool.tile(
                        [128, 1024], BF16, tag="ptd", bufs=3,
                        name=f"pt{ic}_{g[0][0]}",
                    )
                    bal.act += C_GRP_A(fd)
                    nc.scalar.activation(
                        pt[:, 0:fd], sg[:, 0:fd], AF.Exp, bias=0.0, scale=SCALE
                    )
                else:
                    pt = wpool.tile(
                        [128, 1024], F8, tag="pt", bufs=5,
                        name=f"pt{ic}_{g[0][0]}",
                    )
                    eng = bal.pick2([("act", C_GRP_A(fd)), ("dve", C_GRP_D(fd))])
                    if eng == "act":
                        nc.scalar.activation(
                            pt[:, 0:fd], sg[:, 0:fd], AF.Exp, bias=0.0, scale=SCALE
                        )
                    else:
                        nc.vector.tensor_scalar(
                            pt[:, 0:fd].bitcast(I8), sg[:, 0:fd], EA8, EB8,
                            ALU.mult, ALU.add,
                        )
                for _jt, _i_lo, _off, _w, d, _ro in g:
                    if d is not None:
                        bal.gp += C_AFF_G
                        nc.gpsimd.affine_select(
                            out=pt[:, d : d + 128],
                            in_=pt[:, d : d + 128],
                            compare_op=ALU.is_ge,
                            fill=0.0,
                            base=0,
                            pattern=[[1, 128]],
                            channel_multiplier=-1,
                        )
                for kind, cc in bg:
                    if kind == "pq":
                        proj_q(cc)
                    elif kind == "pk":
                        proj_k(cc)
                    else:
                        v_block(cc)
                pendq.append((ic, g, pt, outT_cur, last))

            for ent in pendq:
                flush_pending(ent)

    import concourse.mybir as mybir
    _split_excess_waits(nc, mybir, limit=1)
    if os.environ.get("KERNEL_DEBUG_BAL"):
        print(f"balancer: act={bal.act/1000:.1f}us dve={bal.dve/1000:.1f}us gp={bal.gp/1000:.1f}us")
    return nc


def _get_nc():
    if "nc" not in _CACHE:
        _CACHE["nc"] = _build_nc()
    return _CACHE["nc"]


def kernel(x_text_emb, Wq, Wk, Wv, freqs_cos, freqs_sin, x_latex_mask):
    import ml_dtypes
    from concourse.bass_utils import run_bass_kernel_spmd

    bf16 = ml_dtypes.bfloat16
    nc = _get_nc()

    swap = np.arange(HS) ^ 1
    cos2 = np.repeat(np.asarray(freqs_cos, np.float32).T, 2, axis=0)  # [64, T]
    sin2s = np.repeat(np.asarray(freqs_sin, np.float32).T, 2, axis=0)
    sin2s[0::2] *= -1.0
    cosx = np.ascontiguousarray(np.tile(cos2, (2, 1))).astype(bf16)  # [128, T]
    sinx = np.ascontiguousarray(np.tile(sin2s, (2, 1))).astype(bf16)
    Wq = np.asarray(Wq, np.float32)
    Wk = np.asarray(Wk, np.float32)
    Wv = np.asarray(Wv, np.float32)
    w = np.concatenate(
        [Wq, Wq, Wq[:, swap], Wq[:, swap], Wk, Wk, Wk[:, swap], Wk[:, swap], Wv],
        axis=1,
    ).astype(bf16)
    w = np.ascontiguousarray(w)
    mask01 = np.asarray(x_latex_mask != 0, np.float32).reshape(N_CORES, NT, 128)

    in_maps = []
    for b in range(N_CORES):
        in_maps.append(
            {
                "p_xt": np.ascontiguousarray(
                    np.asarray(x_text_emb[b], np.float32).T
                ).astype(bf16),
                "p_w": w,
                "p_cos": cosx,
                "p_sin": sinx,
                "p_mask": np.ascontiguousarray(mask01[b].T),
            }
        )

    res = run_bass_kernel_spmd(nc, in_maps, core_ids=list(range(N_CORES)))
    outs = []
    for b in range(N_CORES):
        r = np.asarray(res.results[b]["p_out"], np.float32)
        outs.append((r[0:HS, :] / r[HS : HS + 1, :]).T)
    return np.stack(outs, axis=0)
